# revision 45
# baseline (speedup 1.0000x reference)
"""Trainium2 Bass kernel: causal multi-head attention block (B=2,S=2048,H=2048,NH=16,HD=128).

Sharding: 8 cores = DP over batch (2) x TP over head-groups (4 groups of 4 heads).
Each core computes q/k/v projections for its 4 heads, RoPE, causal softmax
attention, and a partial output projection; the host sums the 4 partials per
batch and adds bo.

v21 design notes (evolved from the 414us v3 via NTFF profiling; ~365us now):
  - denominators: exp tiles grouped [128, 4, 512]; DVE quad-sums (2 adds per
    4 k-tiles) + one ones-matmul per quad = 40 PE matmuls instead of 160.
    Masked diagonal sub-regions are DVE-memset to 0 so quad sums stay exact.
  - RoPE entirely on DVE: rotate-half is a partition pair-swap
    (stream_shuffle, 32-lane quadrant permutation) with the sign folded into
    the host-built sin table; no PE permutation matmul, no ACT drain copy.
  - bv is folded into the host-side constant (attn weights sum to 1, so
    attn(v + bv) = attn(v) + bv and the host adds bo + bv@Wo.T exactly);
    v drains are plain ACT/DVE copies.
  - gpsimd does only partition_broadcast; a dummy broadcast at program start
    preloads its custom-op library (~7us) off the critical path, and the
    normalize reads PV from an SBUF copy drained by ACT at pv-stop (frees
    the PV bank early; DVE can read at most one PSUM operand).
  - the o-projection of q-block j runs as 16 (ss,oj) chunks spread EVENLY
    through block j+1's attention steps (from step 6), keeping both the PE
    queue and ACT's exp stream dense; the final block's chunks alternate
    drain engines and borrow idle st PSUM banks so the tail burst is
    matmul-bound. pv lookahead is 4 steps (pv reads SBUF exp-quad tiles, so
    lookahead is not PSUM-bank-limited; 4 gives exp enough slack that pv
    semaphore waits mostly vanish).
  - phase1 schedule is ordered around the ~210GB/s inbound DMA ramp:
    interleaved q0/k0 half-passes match the startup chunk arrival order,
    v waves (which need no new data) fill the later supply holes, and
    narrow tensors (bq/bk/cos/sin) ride the ACT DGE queue so their poor
    descriptor rate never stalls the main stream. 8KB/partition is the
    minimum descriptor size that sustains full DMA rate.
  - PSUM slot arithmetic (7 warmup slots + 4 per pass/wave) lands the last
    v wave on slots 3-6 = attention's late-needed ppv/pdn/pyp banks, so the
    first st matmuls carry no bank anti-dependency on the last v drains.
"""

import math
import os
import sys

import numpy as np

for _p in ("/opt/trn_rl_repo",):
    if _p not in sys.path and os.path.isdir(_p):
        sys.path.insert(0, _p)

import ml_dtypes

import concourse.bass as bass
import concourse.mybir as mybir
import concourse.tile as tile
from concourse import bacc

B, S, H, NH, HD = 2, 2048, 2048, 16, 128
NCORES = 8
HG = 4            # head-groups (TP degree)
HPG = NH // HG    # heads per group = 4
DLOC = HPG * HD   # local d width = 512
FT = H // 128     # 16 f-tiles
SJ = S // 512     # 4 s/q tiles of 512
KT128 = S // 128  # 16 k-tiles of 128
NEG = -1e30
SWAP_MASK = [i ^ 1 for i in range(32)]  # pairwise partition swap per quadrant

F32 = mybir.dt.float32
F32R = mybir.dt.float32r
F16 = mybir.dt.float16
BF16 = mybir.dt.bfloat16
NPBF16 = ml_dtypes.bfloat16


def build_program(mode: str) -> bass.Bass:
    """mode in {'causal', 'full', 'bias'}"""
    import concourse.tile_sem_assignment as tsa

    tsa.NUM_HWDGE_SEMS = 1
    tsa.NUM_SWDGE_GLOBAL_SEMS = 1
    nc = bacc.Bacc()
    # host pre-arranged to SBUF layout: [partition, ...free] contiguous
    xp = nc.dram_tensor("xp", [128, SJ, FT, 512], BF16, kind="ExternalInput")
    wqp = nc.dram_tensor("wqp", [128, FT, DLOC], BF16, kind="ExternalInput")
    wkp = nc.dram_tensor("wkp", [128, FT, DLOC], BF16, kind="ExternalInput")
    wvp = nc.dram_tensor("wvp", [128, FT, DLOC], BF16, kind="ExternalInput")
    wop = nc.dram_tensor("wop", [128, HPG, H], BF16, kind="ExternalInput")
    bqT = nc.dram_tensor("bqT", [128, HPG], F32, kind="ExternalInput")
    bkT = nc.dram_tensor("bkT", [128, HPG], F32, kind="ExternalInput")
    cosT = nc.dram_tensor("cosT", [HD, S], BF16, kind="ExternalInput")
    # sinT is sign-folded on the host: row 2i holds -sin, row 2i+1 holds +sin,
    # so RoPE needs only a partition pair-swap (DVE stream_shuffle) + mul/add.
    sinT = nc.dram_tensor("sinT", [HD, S], BF16, kind="ExternalInput")
    if mode == "causal":
        dbp = nc.dram_tensor("dbp", [128, 4, 512], F32, kind="ExternalInput")
    elif mode == "bias":
        fbias = nc.dram_tensor("fbias", [S, S], F32, kind="ExternalInput")
    # output tiled: y2[qj, ss, p, o] = y[qj*512 + ss*128 + p, o]
    y2 = nc.dram_tensor("y2", [SJ, 4, 128, H], BF16, kind="ExternalOutput")

    with tile.TileContext(nc) as tc:
        with (
            tc.tile_pool(name="qt", bufs=HPG * SJ) as qt_pool,
            tc.tile_pool(name="kt", bufs=HPG * SJ) as kt_pool,
            tc.tile_pool(name="vt", bufs=KT128) as vt_pool,
            tc.tile_pool(name="consts", bufs=1) as consts,
        ):
            QT = {}  # (h, sj) -> [128, 512] bf16 tile, RoPE'd q^T (pre-scaled)
            KT = {}  # (h, sj) -> [128, 512] bf16 tile, RoPE'd k^T
            VT = {}  # ssub -> [128(s), 512(d)] bf16 tile, v + bias

            ones_sb = consts.tile([128, 1], BF16, tag="ones")
            nc.gpsimd.memset(ones_sb[:], 1.0)
            wz_sb = consts.tile([128, 64], BF16, tag="wz")
            nc.gpsimd.memset(wz_sb[:], 0.0)
            # dummy partition_broadcast right after the memsets: the gpsimd
            # custom-op library load costs ~7us and otherwise lands lazily on
            # the first normalize at the phase1->attention boundary, stalling
            # the strict-FIFO DVE queue behind the OT multiply.
            bc_src = consts.tile([1, 1], F32, tag="bcs")
            nc.vector.memset(bc_src[:], 0.0)
            bc_dst = consts.tile([128, 1], F32, tag="bcd")
            nc.gpsimd.partition_broadcast(bc_dst[:], bc_src[:], channels=128)
            wo_sb = consts.tile([128, HPG, H], BF16, tag="wo")
            db_sb = None
            if mode == "causal":
                db_sb = consts.tile([128, 4, 512], F32, tag="db")

            # ============ Phase 1: Q/K/V projections + RoPE (one x pass) ====
            with (
                tc.tile_pool(name="ps12", bufs=8, space="PSUM") as psum,
                tc.tile_pool(name="wqk", bufs=1) as wqk_pool,
                tc.tile_pool(name="csn", bufs=1) as csn_pool,
                tc.tile_pool(name="xin", bufs=SJ) as xin_pool,
                tc.tile_pool(name="rtmp", bufs=3) as rtmp_pool,
                tc.tile_pool(name="rsb", bufs=3) as rsb_pool,
            ):
                wq_sb = wqk_pool.tile([128, FT, DLOC], BF16, tag="wq")
                wk_sb = wqk_pool.tile([128, FT, DLOC], BF16, tag="wk")
                wv_sb = wqk_pool.tile([128, FT, DLOC], BF16, tag="wv")
                cos_sb = csn_pool.tile([HD, S], BF16, tag="cos")
                sin_sb = csn_pool.tile([HD, S], BF16, tag="sin")
                bq_sb = consts.tile([128, HPG], F32, tag="bq")
                bk_sb = consts.tile([128, HPG], F32, tag="bk")

                # PE warmup: dependency-free matmuls on memset tiles run while
                # the first DMAs stream in, releasing the HAM clock gate
                # (K=8/8 after ~3.4us of activity) so the real q-pass starts
                # at full clock instead of ramping through the cold state.
                # 7 psum slots (not 1) so the phase-1 slot count comes out at
                # 83 before sj3's v tiles: those then land on slots 3-6 =
                # attention's ppv/pdn/pyp banks (needed late), keeping the
                # first st matmuls free of PSUM-bank anti-dependencies on the
                # last v drains.
                warm_tiles = [
                    psum.tile([128, 512], F32, tag="ps", name="warm")
                    for _ in range(7)
                ]
                for wi in range(130):
                    nc.tensor.matmul(
                        warm_tiles[wi % 7][:64, :64],
                        wz_sb[:, 0:64],
                        wz_sb[:, 0:64],
                        start=True,
                        stop=True,
                    )
                warm_rd = rsb_pool.tile([64, 64], BF16, tag="wr", name="wr")
                nc.scalar.copy(warm_rd[:], warm_tiles[129 % 7][:64, :64])

                xts = []
                xt0 = xin_pool.tile([128, FT, 512], BF16, tag="xt", name="xt")
                xts.append(xt0)
                # startup in 8-f-tile halves: 8KB/partition descriptors are
                # the smallest size that sustains full DMA rate (finer chunks
                # fall off the descriptor-rate cliff and throttle the whole
                # startup ramp). wk lands during the q matmuls; wv between
                # xt2 and xt3, right before the first v wave needs it.
                h1s, h2s = slice(0, 8), slice(8, 16)
                # x0 rides the ACT dge queue (issued before the small
                # tensors below, while ACT's instruction queue is empty):
                # both queues spin up and stream in parallel, so the weights
                # (sync) and first activations (ACT) arrive concurrently
                # through the ~7us spin-up + early ramp, and every later
                # sync-queue tensor lands ~2MB earlier.
                nc.sync.dma_start(wq_sb[:, h1s, :], wqp[:, h1s, :])
                nc.scalar.dma_start(xt0[:, h1s, :], xp[:, 0, h1s, :])
                nc.sync.dma_start(wk_sb[:, h1s, :], wkp[:, h1s, :])
                nc.sync.dma_start(wq_sb[:, h2s, :], wqp[:, h2s, :])
                nc.scalar.dma_start(xt0[:, h2s, :], xp[:, 0, h2s, :])
                nc.sync.dma_start(wk_sb[:, h2s, :], wkp[:, h2s, :])
                # narrow/tiny tensors (16B-4KB per partition) go on the ACT
                # DGE queue: their poor descriptor rate would stall the main
                # sync-queue stream mid-ramp. ACT is idle until ~18us so they
                # complete long before their first use.
                nc.scalar.dma_start(bq_sb[:], bqT[:])
                nc.scalar.dma_start(bk_sb[:], bkT[:])
                nc.scalar.dma_start(cos_sb[:], cosT[:])
                nc.scalar.dma_start(sin_sb[:], sinT[:])
                # remaining tiles in need order (16KB/partition each)
                xt1 = xin_pool.tile([128, FT, 512], BF16, tag="xt", name="xt")
                xts.append(xt1)
                nc.sync.dma_start(xt1[:], xp[:, 1])
                xt2 = xin_pool.tile([128, FT, 512], BF16, tag="xt", name="xt")
                xts.append(xt2)
                nc.sync.dma_start(xt2[:], xp[:, 2])
                nc.sync.dma_start(wv_sb[:], wvp[:])
                xt3 = xin_pool.tile([128, FT, 512], BF16, tag="xt", name="xt")
                xts.append(xt3)
                nc.sync.dma_start(xt3[:], xp[:, 3])
                nc.sync.dma_start(wo_sb[:], wop[:])
                if mode == "causal":
                    nc.sync.dma_start(db_sb[:], dbp[:])

                def do_vwave(wave):
                    # 4-bank v wave; drains are plain copies (bv folded into
                    # the host-side bias) split across ACT and DVE
                    vps = [psum.tile([128, 512], F32, tag="ps", name="ps") for _ in range(4)]
                    for ft in range(FT):
                        for i in range(4):
                            ss = wave * 4 + i
                            nc.tensor.matmul(
                                vps[i][:],
                                xts[ss // 4][:, ft, (ss % 4) * 128 : (ss % 4 + 1) * 128],
                                wv_sb[:, ft, :],
                                start=(ft == 0),
                                stop=(ft == FT - 1),
                            )
                    for i in range(4):
                        ss = wave * 4 + i
                        v = vt_pool.tile([128, DLOC], BF16, tag="v", name="v")
                        if i % 2 == 0:
                            nc.scalar.copy(v[:], vps[i][:])
                        else:
                            nc.vector.tensor_copy(v[:], vps[i][:])
                        VT[ss] = v

                def do_pass(sj, w_sb, bias_sb, pool, store):
                    # one projection pass (q or k) with its drains and ropes
                    # issued immediately, so only 4 PSUM banks are held
                    xt = xts[sj]
                    pp = [psum.tile([128, 512], F32, tag="ps", name="ps") for _ in range(HPG)]
                    for ft in range(FT):
                        for h in range(HPG):
                            nc.tensor.matmul(
                                pp[h][:],
                                w_sb[:, ft, h * 128 : (h + 1) * 128],
                                xt[:, ft, :],
                                start=(ft == 0),
                                stop=(ft == FT - 1),
                            )
                    css = cos_sb[:, sj * 512 : (sj + 1) * 512]
                    sss = sin_sb[:, sj * 512 : (sj + 1) * 512]
                    for h in range(HPG):
                        t = pool.tile([128, 512], BF16, tag="t", name="qkt")
                        nc.scalar.activation(
                            t[:],
                            pp[h][:],
                            mybir.ActivationFunctionType.Identity,
                            bias=bias_sb[:, h : h + 1],
                        )
                        # rotate-half = partition pair-swap: DVE
                        # stream_shuffle + sign-folded sin table.
                        swp = rtmp_pool.tile([128, 512], BF16, tag="tmp", name="tmp")
                        nc.vector.stream_shuffle(swp[:], t[:], SWAP_MASK)
                        nc.vector.tensor_mul(swp[:], swp[:], sss)
                        nc.vector.tensor_mul(t[:], t[:], css)
                        nc.vector.tensor_add(t[:], t[:], swp[:])
                        store[(h, sj)] = t

                def do_q(sj):
                    do_pass(sj, wq_sb, bq_sb, qt_pool, QT)

                def do_k(sj):
                    do_pass(sj, wk_sb, bk_sb, kt_pool, KT)

                # schedule: passes ordered so each lands just after its DMA
                # (inbound sustains only ~170-190GB/s); the v waves (which
                # need no new data) fill the spots where x tiles would
                # otherwise stall the q/k passes. The slot arithmetic (7 warm
                # + 4 per pass/wave) puts the last v wave on slots 3-6 =
                # attention's late-needed ppv/pdn/pyp banks, so the first st
                # matmuls have no PSUM anti-dependency on the last v drains.
                # sj0 is DMA-ramp-limited: interleave q/k half-passes in
                # the exact order the startup chunks arrive (wq-h1, x0-h1,
                # wk-h1, wq-h2, x0-h2, wk-h2) so the PE never waits ~15us
                # for the second halves mid-pass.
                qp0 = [psum.tile([128, 512], F32, tag="ps", name="ps") for _ in range(HPG)]
                kp0 = [psum.tile([128, 512], F32, tag="ps", name="ps") for _ in range(HPG)]
                for fts, wp, pp in (
                    (range(0, 8), wq_sb, qp0),
                    (range(0, 8), wk_sb, kp0),
                    (range(8, 16), wq_sb, qp0),
                    (range(8, 16), wk_sb, kp0),
                ):
                    for ft in fts:
                        for h in range(HPG):
                            nc.tensor.matmul(
                                pp[h][:],
                                wp[:, ft, h * 128 : (h + 1) * 128],
                                xts[0][:, ft, :],
                                start=(ft == 0),
                                stop=(ft == FT - 1),
                            )
                css0 = cos_sb[:, 0:512]
                sss0 = sin_sb[:, 0:512]
                for pp, bias_sb, pool, store in (
                    (qp0, bq_sb, qt_pool, QT),
                    (kp0, bk_sb, kt_pool, KT),
                ):
                    for h in range(HPG):
                        t = pool.tile([128, 512], BF16, tag="t", name="qkt")
                        nc.scalar.activation(
                            t[:],
                            pp[h][:],
                            mybir.ActivationFunctionType.Identity,
                            bias=bias_sb[:, h : h + 1],
                        )
                        swp = rtmp_pool.tile([128, 512], BF16, tag="tmp", name="tmp")
                        nc.vector.stream_shuffle(swp[:], t[:], SWAP_MASK)
                        nc.vector.tensor_mul(swp[:], swp[:], sss0)
                        nc.vector.tensor_mul(t[:], t[:], css0)
                        nc.vector.tensor_add(t[:], t[:], swp[:])
                        store[(h, 0)] = t
                do_q(1)
                do_k(1)
                do_q(2)
                do_vwave(0)
                do_k(2)
                do_q(3)
                do_vwave(1)
                do_k(3)
                do_vwave(2)
                do_vwave(3)

            # ============ Phase 3: attention + output projection ============
            with (
                tc.tile_pool(name="pst", bufs=3, space="PSUM") as psum_st,
                tc.tile_pool(name="ppv", bufs=2, space="PSUM") as psum_pv,
                tc.tile_pool(name="pdn", bufs=1, space="PSUM") as psum_dn,
                tc.tile_pool(name="pyp", bufs=2, space="PSUM") as psum_yp,
                tc.tile_pool(name="ex", bufs=3) as exp_pool,
                tc.tile_pool(name="ep", bufs=2) as ep_pool,
                tc.tile_pool(name="es", bufs=2) as es_pool,
                tc.tile_pool(name="ot", bufs=2 * HPG) as ot_pool,
                tc.tile_pool(name="pvs", bufs=2) as pvs_pool,
                tc.tile_pool(name="rc", bufs=4) as rc_pool,
                tc.tile_pool(name="ysb", bufs=4) as y_pool,
                tc.tile_pool(name="fb", bufs=3) as fb_pool,
            ):
                pending_oproj = [None]

                def make_oproj_chunks(qj, OT, tail=False):
                    # 16 chunks of one (ss, oj) accumulation each; emitted one
                    # per attention step of the next q-block so the PE queue
                    # never bursts 64 oproj matmuls (which starved ACT of st
                    # tiles and stalled the exp->pv chain in v3/v4).
                    # In tail mode (the last q-block, running after all
                    # attention) ACT is free: alternate drain engines and
                    # borrow the idle st PSUM banks so the burst is matmul-
                    # bound instead of drain-bound.
                    ysbs = {}

                    def chunk(ss, oj, ci):
                        def emit():
                            if oj == 0:
                                ysbs[ss] = y_pool.tile(
                                    [128, H], BF16, tag="y", name="y"
                                )
                            ysb = ysbs[ss]
                            if tail and ci % 2 == 1:
                                yp = psum_st.tile([128, 512], F32, tag="st", name="yp")
                            else:
                                yp = psum_yp.tile([128, 512], F32, tag="yp", name="yp")
                            for dt in range(HPG):
                                nc.tensor.matmul(
                                    yp[:],
                                    OT[dt][:, ss * 128 : (ss + 1) * 128],
                                    wo_sb[:, dt, oj * 512 : (oj + 1) * 512],
                                    start=(dt == 0),
                                    stop=(dt == HPG - 1),
                                )
                            dst = ysb[:, oj * 512 : (oj + 1) * 512]
                            # mid-attention: DVE-only drains (ACT drains here
                            # delayed the exp stream and stalled exp->pv)
                            if tail and ci % 2 == 1:
                                nc.scalar.copy(dst, yp[:])
                            else:
                                nc.vector.tensor_copy(dst, yp[:])
                            if oj == 1:
                                nc.sync.dma_start(
                                    y2[qj, ss, :, 0:1024], ysb[:, 0:1024]
                                )
                            elif oj == 3:
                                nc.sync.dma_start(
                                    y2[qj, ss, :, 1024:2048], ysb[:, 1024:2048]
                                )
                        return emit

                    return [
                        chunk(ss, oj, 4 * ss + oj)
                        for ss in range(4)
                        for oj in range(4)
                    ]

                for qj in range(SJ):
                    kmax = 4 * qj + 4 if mode == "causal" else KT128
                    nquad = kmax // 4
                    OT = {}
                    PV = {}
                    PVS = {}
                    DN = {}
                    RC = {}
                    EQ = {}  # (h, quad) -> [128, 4, 512] bf16 exp tile group

                    def _off(kj):
                        a = kj - 4 * qj
                        return 128 * a if (mode == "causal" and a > 0) else 0

                    def _issue_st(h, kj):
                        off = _off(kj)
                        a = kj % 4
                        if a == 0:
                            EQ[(h, kj // 4)] = exp_pool.tile(
                                [128, 4, 512], BF16, tag="e", name="e"
                            )
                        eq = EQ[(h, kj // 4)]
                        if off > 0:
                            # zero the masked region so the quad sums stay
                            # correct (DVE: keeps gpsimd out of attention so
                            # no custom-op library swaps happen there)
                            nc.vector.memset(eq[:, a, 0:off], 0.0)
                        st = psum_st.tile([128, 512], F32, tag="st", name="st")
                        nc.tensor.matmul(
                            st[:, off:],
                            KT[(h, kj // 4)][:, (kj % 4) * 128 : (kj % 4 + 1) * 128],
                            QT[(h, qj)][:, off:],
                            start=True,
                            stop=True,
                        )
                        ad = kj - 4 * qj
                        if mode == "causal" and ad >= 0:
                            nc.vector.tensor_add(
                                st[:, off : off + 128],
                                st[:, off : off + 128],
                                db_sb[:, ad, off : off + 128],
                            )
                        elif mode == "bias":
                            fbt = fb_pool.tile([128, 512], F32, tag="fb", name="fb")
                            nc.sync.dma_start(
                                fbt[:],
                                fbias[
                                    kj * 128 : (kj + 1) * 128,
                                    qj * 512 : (qj + 1) * 512,
                                ],
                            )
                            nc.vector.tensor_add(st[:], st[:], fbt[:])
                        nc.scalar.activation(
                            eq[:, a, off:], st[:, off:],
                            mybir.ActivationFunctionType.Exp,
                        )

                    def _normalize(i):
                        # 1/denom broadcast across partitions on GPSIMD.
                        # gpsimd runs nothing else in attention (memsets moved
                        # to DVE) so the custom-op library stays loaded, and
                        # the PV bank was already freed by the ACT drain, so
                        # this latency only gates the oproj chunks which have
                        # >= 6 steps of slack.
                        rcb = rc_pool.tile([128, 512], F32, tag="rcb", name="rcb")
                        nc.gpsimd.partition_broadcast(rcb[:], RC[i][:], channels=128)
                        ot = ot_pool.tile([128, 512], BF16, tag="ot", name="ot")
                        nc.vector.tensor_mul(ot[:], PVS[i][:], rcb[:])
                        OT[i] = ot

                    def _issue_pvdn(h, kj):
                        off = _off(kj)
                        a = kj % 4
                        eq = EQ[(h, kj // 4)]
                        if kj == 0:
                            PV[h] = psum_pv.tile([128, 512], F32, tag="pv", name="pv")
                            DN[h] = psum_dn.tile([1, 512], F32, tag="dn", name="dn")
                        nc.tensor.matmul(
                            PV[h][:, off:],
                            VT[kj][:, h * 128 : (h + 1) * 128],
                            eq[:, a, off:],
                            start=(kj == 0),
                            stop=(kj == kmax - 1),
                        )
                        if a == 3:
                            # quad denominator: 2 DVE adds + 1 matmul instead
                            # of 4 ones-matmuls
                            quad = kj // 4
                            p2 = ep_pool.tile([128, 2, 512], BF16, tag="p2", name="p2")
                            nc.vector.tensor_add(p2[:], eq[:, 0:2, :], eq[:, 2:4, :])
                            sq = es_pool.tile([128, 512], BF16, tag="sq", name="sq")
                            nc.vector.tensor_add(sq[:], p2[:, 0, :], p2[:, 1, :])
                            nc.tensor.matmul(
                                DN[h][:],
                                ones_sb[:],
                                sq[:],
                                start=(quad == 0),
                                stop=(quad == nquad - 1),
                            )
                        if kj == kmax - 1:
                            # drain PV to SBUF on ACT right at pv-stop: frees
                            # the PV bank early and gives the normalize mul an
                            # SBUF operand (DVE reads at most one PSUM input)
                            pvsb = pvs_pool.tile([128, 512], BF16, tag="pvs", name="pvs")
                            nc.scalar.copy(pvsb[:], PV[h][:])
                            PVS[h] = pvsb
                            rcf = rc_pool.tile([1, 512], F32, tag="rcf", name="rcf")
                            nc.vector.reciprocal_approx_fast(rcf[:], DN[h][:])
                            RC[h] = rcf
                            if h > 0:
                                _normalize(h - 1)
                            if h == HPG - 1:
                                _normalize(h)

                    seq = [(h, kj) for h in range(HPG) for kj in range(kmax)]
                    LOOK = 4
                    chunks = pending_oproj[0] or []
                    pending_oproj[0] = None
                    # previous q-block's o-projection chunks, spread EVENLY
                    # over steps 6..len-1 (start at 6 so the first chunk,
                    # which waits on the previous block's last OT normalize,
                    # never blocks the in-order PE queue ahead of these sts).
                    # Even spacing keeps ACT (exp, ~690ns/tile) from falling
                    # behind in the chunk-free stretch at the end of each
                    # block, where a bare step is only ~550ns of PE work.
                    emit_at = {}
                    if chunks:
                        span = len(seq) - 6
                        for k in range(len(chunks)):
                            emit_at[6 + (k * span) // len(chunks)] = k
                    ci = 0
                    for i, (h, kj) in enumerate(seq):
                        _issue_st(h, kj)
                        if i >= LOOK:
                            _issue_pvdn(*seq[i - LOOK])
                        if i in emit_at and ci < len(chunks):
                            chunks[ci]()
                            ci += 1
                    for i in range(len(seq) - LOOK, len(seq)):
                        _issue_pvdn(*seq[i])
                    while ci < len(chunks):
                        chunks[ci]()
                        ci += 1
                    pending_oproj[0] = make_oproj_chunks(qj, OT, tail=(qj == SJ - 1))
                for ch in pending_oproj[0]:
                    ch()
    nc.compile()
    return nc


_PROGRAM_CACHE = {}


def _get_program(mode):
    if mode not in _PROGRAM_CACHE:
        _PROGRAM_CACHE[mode] = build_program(mode)
    return _PROGRAM_CACHE[mode]


def _detect_mode(attn_mask):
    m = np.asarray(attn_mask).reshape(S, S)
    if (m == np.tril(np.ones((S, S), m.dtype))).all():
        return "causal"
    if (m != 0).all():
        return "full"
    return "bias"


def _diag_bias():
    # [128(p), 4(a), 512(t)]: 0 where 128a+p <= t else -1e30
    a = np.arange(4)[None, :, None]
    p = np.arange(128)[:, None, None]
    t = np.arange(512)[None, None, :]
    return np.where(128 * a + p <= t, 0.0, NEG).astype(np.float32)


def _bf16(a):
    return np.ascontiguousarray(a).astype(NPBF16)


def _prep_w(wT):
    # [H, DLOC] -> [128, FT, DLOC] with [p, ft, d] = wT[ft*128+p, d]
    return np.ascontiguousarray(wT.reshape(FT, 128, DLOC).transpose(1, 0, 2))


def kernel(**inputs) -> np.ndarray:
    from concourse.bass_utils import run_bass_kernel_spmd

    x = np.asarray(inputs["x"], np.float32)
    fcos = np.asarray(inputs["fcos"], np.float32)
    fsin = np.asarray(inputs["fsin"], np.float32)
    Wq, bq = np.asarray(inputs["Wq"], np.float32), np.asarray(inputs["bq"], np.float32)
    Wk, bk = np.asarray(inputs["Wk"], np.float32), np.asarray(inputs["bk"], np.float32)
    Wv, bv = np.asarray(inputs["Wv"], np.float32), np.asarray(inputs["bv"], np.float32)
    Wo, bo = np.asarray(inputs["Wo"], np.float32), np.asarray(inputs["bo"], np.float32)
    attn_mask = inputs["attn_mask"]

    mode = _detect_mode(attn_mask)
    nc = _get_program(mode)

    sc = 1.0 / math.sqrt(HD)
    sinF = fsin.T.copy()  # [HD, S]
    sinF[0::2, :] *= -1.0  # rope[2i] = t[2i]*cos - t[2i+1]*sin
    shared = {
        "cosT": _bf16(fcos.T),
        "sinT": _bf16(sinF),
    }
    if mode == "causal":
        shared["dbp"] = _diag_bias()
    elif mode == "bias":
        m = np.asarray(attn_mask).reshape(S, S)
        shared["fbias"] = np.ascontiguousarray(
            np.where(m.T == 0, NEG, 0.0).astype(np.float32)
        )

    in_maps = []
    for c in range(NCORES):
        b, hg = divmod(c, HG)
        rows = slice(DLOC * hg, DLOC * (hg + 1))
        xT = x[b].T  # [H, S]
        # [128, SJ, FT, 512]: [p, sj, ft, s] = xT[ft*128+p, sj*512+s]
        xprep = xT.reshape(FT, 128, SJ, 512).transpose(1, 2, 0, 3)
        woT = Wo[:, rows].T  # [DLOC, H]
        wo_prep = woT.reshape(HPG, 128, H).transpose(1, 0, 2)
        in_maps.append(
            {
                "xp": _bf16(xprep),
                "wqp": _bf16(_prep_w((Wq[rows] * sc).T)),
                "wkp": _bf16(_prep_w(Wk[rows].T)),
                "wvp": _bf16(_prep_w(Wv[rows].T)),
                "wop": _bf16(wo_prep),
                "bqT": np.ascontiguousarray((bq[rows] * sc).reshape(HPG, 128).T),
                "bkT": np.ascontiguousarray(bk[rows].reshape(HPG, 128).T),
                **shared,
            }
        )

    trace = bool(int(os.environ.get("KERNEL_TRACE", "0")))
    res = run_bass_kernel_spmd(nc, in_maps, list(range(NCORES)), trace=trace)
    if trace and res.exec_time_ns is not None:
        print(f"HW exec time: {res.exec_time_ns} ns")
        globals()["LAST_EXEC_NS"] = res.exec_time_ns
        globals()["LAST_RESULTS"] = res

    out = np.zeros((B, S, H), np.float32)
    for c in range(NCORES):
        yt = np.asarray(res.results[c]["y2"]).astype(np.float32)  # [SJ,4,128,H]
        out[c // HG] += yt.reshape(S, H)
    # bv is folded here: attn weights sum to 1, so attn(v + bv) = attn(v) + bv
    # and the output projection adds the constant row bv @ Wo.T exactly.
    out += bo + bv @ Wo.T
    return out


# revision 47
# speedup vs baseline: 1.0179x; 1.0179x over previous
"""Trainium2 Bass kernel: causal multi-head attention block (B=2,S=2048,H=2048,NH=16,HD=128).

Sharding: 8 cores = DP over batch (2) x TP over head-groups (4 groups of 4 heads).
Each core computes q/k/v projections for its 4 heads, RoPE, causal softmax
attention, and a partial output projection; the host sums the 4 partials per
batch and adds bo.

v21 design notes (evolved from the 414us v3 via NTFF profiling; ~365us now):
  - denominators: exp tiles grouped [128, 4, 512]; DVE quad-sums (2 adds per
    4 k-tiles) + one ones-matmul per quad = 40 PE matmuls instead of 160.
    Masked diagonal sub-regions are DVE-memset to 0 so quad sums stay exact.
  - RoPE entirely on DVE: rotate-half is a partition pair-swap
    (stream_shuffle, 32-lane quadrant permutation) with the sign folded into
    the host-built sin table; no PE permutation matmul, no ACT drain copy.
  - bv is folded into the host-side constant (attn weights sum to 1, so
    attn(v + bv) = attn(v) + bv and the host adds bo + bv@Wo.T exactly);
    v drains are plain ACT/DVE copies.
  - gpsimd does only partition_broadcast; a dummy broadcast at program start
    preloads its custom-op library (~7us) off the critical path, and the
    normalize reads PV from an SBUF copy drained by ACT at pv-stop (frees
    the PV bank early; DVE can read at most one PSUM operand).
  - the o-projection of q-block j runs as 16 (ss,oj) chunks spread EVENLY
    through block j+1's attention steps (from step 6), keeping both the PE
    queue and ACT's exp stream dense; the final block's chunks alternate
    drain engines and borrow idle st PSUM banks so the tail burst is
    matmul-bound. pv lookahead is 4 steps (pv reads SBUF exp-quad tiles, so
    lookahead is not PSUM-bank-limited; 4 gives exp enough slack that pv
    semaphore waits mostly vanish).
  - phase1 schedule is ordered around the ~210GB/s inbound DMA ramp:
    interleaved q0/k0 half-passes match the startup chunk arrival order,
    v waves (which need no new data) fill the later supply holes, and
    narrow tensors (bq/bk/cos/sin) ride the ACT DGE queue so their poor
    descriptor rate never stalls the main stream. 8KB/partition is the
    minimum descriptor size that sustains full DMA rate.
  - PSUM slot arithmetic (7 warmup slots + 4 per pass/wave) lands the last
    v wave on slots 3-6 = attention's late-needed ppv/pdn/pyp banks, so the
    first st matmuls carry no bank anti-dependency on the last v drains.
"""

import math
import os
import sys

import numpy as np

for _p in ("/opt/trn_rl_repo",):
    if _p not in sys.path and os.path.isdir(_p):
        sys.path.insert(0, _p)

import ml_dtypes

import concourse.bass as bass
import concourse.mybir as mybir
import concourse.tile as tile
from concourse import bacc

B, S, H, NH, HD = 2, 2048, 2048, 16, 128
NCORES = 8
HG = 4            # head-groups (TP degree)
HPG = NH // HG    # heads per group = 4
DLOC = HPG * HD   # local d width = 512
FT = H // 128     # 16 f-tiles
SJ = S // 512     # 4 s/q tiles of 512
KT128 = S // 128  # 16 k-tiles of 128
NEG = -1e30
SWAP_MASK = [i ^ 1 for i in range(32)]  # pairwise partition swap per quadrant

F32 = mybir.dt.float32
F32R = mybir.dt.float32r
F16 = mybir.dt.float16
BF16 = mybir.dt.bfloat16
NPBF16 = ml_dtypes.bfloat16


def build_program(mode: str) -> bass.Bass:
    """mode in {'causal', 'full', 'bias'}"""
    import concourse.tile_sem_assignment as tsa

    tsa.NUM_HWDGE_SEMS = 1
    tsa.NUM_SWDGE_GLOBAL_SEMS = 1
    nc = bacc.Bacc()
    # host pre-arranged to SBUF layout: [partition, ...free] contiguous
    xp = nc.dram_tensor("xp", [128, SJ, FT, 512], BF16, kind="ExternalInput")
    wqp = nc.dram_tensor("wqp", [128, FT, DLOC], BF16, kind="ExternalInput")
    wkp = nc.dram_tensor("wkp", [128, FT, DLOC], BF16, kind="ExternalInput")
    wvp = nc.dram_tensor("wvp", [128, FT, DLOC], BF16, kind="ExternalInput")
    wop = nc.dram_tensor("wop", [128, HPG, H], BF16, kind="ExternalInput")
    bqT = nc.dram_tensor("bqT", [128, HPG], F32, kind="ExternalInput")
    bkT = nc.dram_tensor("bkT", [128, HPG], F32, kind="ExternalInput")
    cosT = nc.dram_tensor("cosT", [HD, S], BF16, kind="ExternalInput")
    # sinT is sign-folded on the host: row 2i holds -sin, row 2i+1 holds +sin,
    # so RoPE needs only a partition pair-swap (DVE stream_shuffle) + mul/add.
    sinT = nc.dram_tensor("sinT", [HD, S], BF16, kind="ExternalInput")
    if mode == "causal":
        dbp = nc.dram_tensor("dbp", [128, 4, 512], F32, kind="ExternalInput")
    elif mode == "bias":
        fbias = nc.dram_tensor("fbias", [S, S], F32, kind="ExternalInput")
    # output tiled: y2[qj, ss, p, o] = y[qj*512 + ss*128 + p, o]
    y2 = nc.dram_tensor("y2", [SJ, 4, 128, H], BF16, kind="ExternalOutput")

    with tile.TileContext(nc) as tc:
        with (
            tc.tile_pool(name="qt", bufs=HPG * SJ) as qt_pool,
            tc.tile_pool(name="kt", bufs=HPG * SJ) as kt_pool,
            tc.tile_pool(name="vt", bufs=KT128) as vt_pool,
            tc.tile_pool(name="consts", bufs=1) as consts,
        ):
            QT = {}  # (h, sj) -> [128, 512] bf16 tile, RoPE'd q^T (pre-scaled)
            KT = {}  # (h, sj) -> [128, 512] bf16 tile, RoPE'd k^T
            VT = {}  # ssub -> [128(s), 512(d)] bf16 tile, v + bias

            ones_sb = consts.tile([128, 1], BF16, tag="ones")
            nc.gpsimd.memset(ones_sb[:], 1.0)
            wz_sb = consts.tile([128, 64], BF16, tag="wz")
            nc.gpsimd.memset(wz_sb[:], 0.0)
            # dummy partition_broadcast right after the memsets: the gpsimd
            # custom-op library load costs ~7us and otherwise lands lazily on
            # the first normalize at the phase1->attention boundary, stalling
            # the strict-FIFO DVE queue behind the OT multiply.
            bc_src = consts.tile([1, 1], F32, tag="bcs")
            nc.vector.memset(bc_src[:], 0.0)
            bc_dst = consts.tile([128, 1], F32, tag="bcd")
            nc.gpsimd.partition_broadcast(bc_dst[:], bc_src[:], channels=128)
            wo_sb = consts.tile([128, HPG, H], BF16, tag="wo")
            db_sb = None
            if mode == "causal":
                db_sb = consts.tile([128, 4, 512], F32, tag="db")

            # ============ Phase 1: Q/K/V projections + RoPE (one x pass) ====
            with (
                tc.tile_pool(name="ps12", bufs=8, space="PSUM") as psum,
                tc.tile_pool(name="wqk", bufs=1) as wqk_pool,
                tc.tile_pool(name="csn", bufs=1) as csn_pool,
                tc.tile_pool(name="xin", bufs=SJ) as xin_pool,
                tc.tile_pool(name="rtmp", bufs=3) as rtmp_pool,
                tc.tile_pool(name="rsb", bufs=3) as rsb_pool,
            ):
                wq_sb = wqk_pool.tile([128, FT, DLOC], BF16, tag="wq")
                wk_sb = wqk_pool.tile([128, FT, DLOC], BF16, tag="wk")
                wv_sb = wqk_pool.tile([128, FT, DLOC], BF16, tag="wv")
                cos_sb = csn_pool.tile([HD, S], BF16, tag="cos")
                sin_sb = csn_pool.tile([HD, S], BF16, tag="sin")
                bq_sb = consts.tile([128, HPG], F32, tag="bq")
                bk_sb = consts.tile([128, HPG], F32, tag="bk")

                # PE warmup: dependency-free matmuls on memset tiles run while
                # the first DMAs stream in, releasing the HAM clock gate
                # (K=8/8 after ~3.4us of activity) so the real q-pass starts
                # at full clock instead of ramping through the cold state.
                # 7 psum slots (not 1) so the phase-1 slot count comes out at
                # 83 before sj3's v tiles: those then land on slots 3-6 =
                # attention's ppv/pdn/pyp banks (needed late), keeping the
                # first st matmuls free of PSUM-bank anti-dependencies on the
                # last v drains.
                warm_tiles = [
                    psum.tile([128, 512], F32, tag="ps", name="warm")
                    for _ in range(7)
                ]
                for wi in range(130):
                    nc.tensor.matmul(
                        warm_tiles[wi % 7][:64, :64],
                        wz_sb[:, 0:64],
                        wz_sb[:, 0:64],
                        start=True,
                        stop=True,
                    )
                warm_rd = rsb_pool.tile([64, 64], BF16, tag="wr", name="wr")
                nc.scalar.copy(warm_rd[:], warm_tiles[129 % 7][:64, :64])

                xts = []
                xt0 = xin_pool.tile([128, FT, 512], BF16, tag="xt", name="xt")
                xts.append(xt0)
                # startup in 8-f-tile halves: 8KB/partition descriptors are
                # the smallest size that sustains full DMA rate (finer chunks
                # fall off the descriptor-rate cliff and throttle the whole
                # startup ramp). wk lands during the q matmuls; wv between
                # xt2 and xt3, right before the first v wave needs it.
                h1s, h2s = slice(0, 8), slice(8, 16)
                # x0 rides the ACT dge queue (issued before the small
                # tensors below, while ACT's instruction queue is empty):
                # both queues spin up and stream in parallel, so the weights
                # (sync) and first activations (ACT) arrive concurrently
                # through the ~7us spin-up + early ramp, and every later
                # sync-queue tensor lands ~2MB earlier.
                nc.sync.dma_start(wq_sb[:, h1s, :], wqp[:, h1s, :])
                nc.scalar.dma_start(xt0[:, h1s, :], xp[:, 0, h1s, :])
                nc.sync.dma_start(wk_sb[:, h1s, :], wkp[:, h1s, :])
                nc.sync.dma_start(wq_sb[:, h2s, :], wqp[:, h2s, :])
                nc.scalar.dma_start(xt0[:, h2s, :], xp[:, 0, h2s, :])
                nc.sync.dma_start(wk_sb[:, h2s, :], wkp[:, h2s, :])
                # narrow/tiny tensors (16B-4KB per partition) go on the ACT
                # DGE queue: their poor descriptor rate would stall the main
                # sync-queue stream mid-ramp. ACT is idle until ~18us so they
                # complete long before their first use.
                nc.scalar.dma_start(bq_sb[:], bqT[:])
                nc.scalar.dma_start(bk_sb[:], bkT[:])
                nc.scalar.dma_start(cos_sb[:], cosT[:])
                nc.scalar.dma_start(sin_sb[:], sinT[:])
                # remaining tiles in need order (16KB/partition each)
                xt1 = xin_pool.tile([128, FT, 512], BF16, tag="xt", name="xt")
                xts.append(xt1)
                nc.sync.dma_start(xt1[:], xp[:, 1])
                xt2 = xin_pool.tile([128, FT, 512], BF16, tag="xt", name="xt")
                xts.append(xt2)
                nc.sync.dma_start(xt2[:], xp[:, 2])
                nc.sync.dma_start(wv_sb[:], wvp[:])
                xt3 = xin_pool.tile([128, FT, 512], BF16, tag="xt", name="xt")
                xts.append(xt3)
                nc.sync.dma_start(xt3[:], xp[:, 3])
                nc.sync.dma_start(wo_sb[:], wop[:])
                if mode == "causal":
                    nc.sync.dma_start(db_sb[:], dbp[:])

                def do_vwave(wave):
                    # 4-bank v wave; drains are plain copies (bv folded into
                    # the host-side bias) split across ACT and DVE
                    vps = [psum.tile([128, 512], F32, tag="ps", name="ps") for _ in range(4)]
                    for ft in range(FT):
                        for i in range(4):
                            ss = wave * 4 + i
                            nc.tensor.matmul(
                                vps[i][:],
                                xts[ss // 4][:, ft, (ss % 4) * 128 : (ss % 4 + 1) * 128],
                                wv_sb[:, ft, :],
                                start=(ft == 0),
                                stop=(ft == FT - 1),
                            )
                    for i in range(4):
                        ss = wave * 4 + i
                        v = vt_pool.tile([128, DLOC], BF16, tag="v", name="v")
                        if i % 2 == 0:
                            nc.scalar.copy(v[:], vps[i][:])
                        else:
                            nc.vector.tensor_copy(v[:], vps[i][:])
                        VT[ss] = v

                def do_pass(sj, w_sb, bias_sb, pool, store):
                    # one projection pass (q or k) with its drains and ropes
                    # issued immediately, so only 4 PSUM banks are held
                    xt = xts[sj]
                    pp = [psum.tile([128, 512], F32, tag="ps", name="ps") for _ in range(HPG)]
                    for ft in range(FT):
                        for h in range(HPG):
                            nc.tensor.matmul(
                                pp[h][:],
                                w_sb[:, ft, h * 128 : (h + 1) * 128],
                                xt[:, ft, :],
                                start=(ft == 0),
                                stop=(ft == FT - 1),
                            )
                    css = cos_sb[:, sj * 512 : (sj + 1) * 512]
                    sss = sin_sb[:, sj * 512 : (sj + 1) * 512]
                    for h in range(HPG):
                        t = pool.tile([128, 512], BF16, tag="t", name="qkt")
                        nc.scalar.activation(
                            t[:],
                            pp[h][:],
                            mybir.ActivationFunctionType.Identity,
                            bias=bias_sb[:, h : h + 1],
                        )
                        # rotate-half = partition pair-swap: DVE
                        # stream_shuffle + sign-folded sin table.
                        swp = rtmp_pool.tile([128, 512], BF16, tag="tmp", name="tmp")
                        nc.vector.stream_shuffle(swp[:], t[:], SWAP_MASK)
                        nc.vector.tensor_mul(swp[:], swp[:], sss)
                        nc.vector.tensor_mul(t[:], t[:], css)
                        nc.vector.tensor_add(t[:], t[:], swp[:])
                        store[(h, sj)] = t

                def do_q(sj):
                    do_pass(sj, wq_sb, bq_sb, qt_pool, QT)

                def do_k(sj):
                    do_pass(sj, wk_sb, bk_sb, kt_pool, KT)

                # schedule: passes ordered so each lands just after its DMA
                # (inbound sustains only ~170-190GB/s); the v waves (which
                # need no new data) fill the spots where x tiles would
                # otherwise stall the q/k passes. The slot arithmetic (7 warm
                # + 4 per pass/wave) puts the last v wave on slots 3-6 =
                # attention's late-needed ppv/pdn/pyp banks, so the first st
                # matmuls have no PSUM anti-dependency on the last v drains.
                # sj0 is DMA-ramp-limited: interleave q/k half-passes in
                # the exact order the startup chunks arrive (wq-h1, x0-h1,
                # wk-h1, wq-h2, x0-h2, wk-h2) so the PE never waits ~15us
                # for the second halves mid-pass.
                qp0 = [psum.tile([128, 512], F32, tag="ps", name="ps") for _ in range(HPG)]
                kp0 = [psum.tile([128, 512], F32, tag="ps", name="ps") for _ in range(HPG)]
                for fts, wp, pp in (
                    (range(0, 8), wq_sb, qp0),
                    (range(0, 8), wk_sb, kp0),
                    (range(8, 16), wq_sb, qp0),
                    (range(8, 16), wk_sb, kp0),
                ):
                    for ft in fts:
                        for h in range(HPG):
                            nc.tensor.matmul(
                                pp[h][:],
                                wp[:, ft, h * 128 : (h + 1) * 128],
                                xts[0][:, ft, :],
                                start=(ft == 0),
                                stop=(ft == FT - 1),
                            )
                css0 = cos_sb[:, 0:512]
                sss0 = sin_sb[:, 0:512]
                for pp, bias_sb, pool, store in (
                    (qp0, bq_sb, qt_pool, QT),
                    (kp0, bk_sb, kt_pool, KT),
                ):
                    for h in range(HPG):
                        t = pool.tile([128, 512], BF16, tag="t", name="qkt")
                        nc.scalar.activation(
                            t[:],
                            pp[h][:],
                            mybir.ActivationFunctionType.Identity,
                            bias=bias_sb[:, h : h + 1],
                        )
                        swp = rtmp_pool.tile([128, 512], BF16, tag="tmp", name="tmp")
                        nc.vector.stream_shuffle(swp[:], t[:], SWAP_MASK)
                        nc.vector.tensor_mul(swp[:], swp[:], sss0)
                        nc.vector.tensor_mul(t[:], t[:], css0)
                        nc.vector.tensor_add(t[:], t[:], swp[:])
                        store[(h, 0)] = t
                do_q(1)
                do_k(1)
                do_q(2)
                do_vwave(0)
                do_k(2)
                do_q(3)
                do_vwave(1)
                do_k(3)
                do_vwave(2)
                do_vwave(3)

            # ============ Phase 3: attention + output projection ============
            with (
                tc.tile_pool(name="pst", bufs=3, space="PSUM") as psum_st,
                tc.tile_pool(name="ppv", bufs=2, space="PSUM") as psum_pv,
                tc.tile_pool(name="pdn", bufs=1, space="PSUM") as psum_dn,
                tc.tile_pool(name="pyp", bufs=2, space="PSUM") as psum_yp,
                tc.tile_pool(name="ex", bufs=3) as exp_pool,
                tc.tile_pool(name="ep", bufs=2) as ep_pool,
                tc.tile_pool(name="es", bufs=2) as es_pool,
                tc.tile_pool(name="ot", bufs=2 * HPG) as ot_pool,
                tc.tile_pool(name="pvs", bufs=2) as pvs_pool,
                tc.tile_pool(name="rc", bufs=4) as rc_pool,
                tc.tile_pool(name="ysb", bufs=4) as y_pool,
                tc.tile_pool(name="fb", bufs=3) as fb_pool,
            ):
                pending_oproj = [None]

                def make_oproj_chunks(qj, OT, tail=False):
                    # 16 chunks of one (ss, oj) accumulation each; emitted one
                    # per attention step of the next q-block so the PE queue
                    # never bursts 64 oproj matmuls (which starved ACT of st
                    # tiles and stalled the exp->pv chain in v3/v4).
                    # In tail mode (the last q-block, running after all
                    # attention) ACT is free: alternate drain engines and
                    # borrow the idle st PSUM banks so the burst is matmul-
                    # bound instead of drain-bound.
                    ysbs = {}

                    def chunk(ss, oj, ci):
                        def emit():
                            if oj == 0:
                                ysbs[ss] = y_pool.tile(
                                    [128, H], BF16, tag="y", name="y"
                                )
                            ysb = ysbs[ss]
                            if tail and ci % 2 == 1:
                                yp = psum_st.tile([128, 512], F32, tag="st", name="yp")
                            else:
                                yp = psum_yp.tile([128, 512], F32, tag="yp", name="yp")
                            for dt in range(HPG):
                                nc.tensor.matmul(
                                    yp[:],
                                    OT[dt][:, ss * 128 : (ss + 1) * 128],
                                    wo_sb[:, dt, oj * 512 : (oj + 1) * 512],
                                    start=(dt == 0),
                                    stop=(dt == HPG - 1),
                                )
                            dst = ysb[:, oj * 512 : (oj + 1) * 512]
                            # mid-attention: DVE-only drains (ACT drains here
                            # delayed the exp stream and stalled exp->pv)
                            if tail and ci % 2 == 1:
                                nc.scalar.copy(dst, yp[:])
                            else:
                                nc.vector.tensor_copy(dst, yp[:])
                            if oj == 1:
                                nc.sync.dma_start(
                                    y2[qj, ss, :, 0:1024], ysb[:, 0:1024]
                                )
                            elif oj == 3:
                                nc.sync.dma_start(
                                    y2[qj, ss, :, 1024:2048], ysb[:, 1024:2048]
                                )
                        return emit

                    return [
                        chunk(ss, oj, 4 * ss + oj)
                        for ss in range(4)
                        for oj in range(4)
                    ]

                for qj in range(SJ):
                    kmax = 4 * qj + 4 if mode == "causal" else KT128
                    nquad = kmax // 4
                    OT = {}
                    PV = {}
                    PVS = {}
                    DN = {}
                    RC = {}
                    EQ = {}  # (h, quad) -> [128, 4, 512] bf16 exp tile group

                    def _off(kj):
                        a = kj - 4 * qj
                        return 128 * a if (mode == "causal" and a > 0) else 0

                    def _issue_st(h, kj):
                        off = _off(kj)
                        a = kj % 4
                        if a == 0:
                            EQ[(h, kj // 4)] = exp_pool.tile(
                                [128, 4, 512], BF16, tag="e", name="e"
                            )
                        eq = EQ[(h, kj // 4)]
                        if off > 0:
                            # zero the masked region so the quad sums stay
                            # correct (DVE: keeps gpsimd out of attention so
                            # no custom-op library swaps happen there)
                            nc.vector.memset(eq[:, a, 0:off], 0.0)
                        st = psum_st.tile([128, 512], F32, tag="st", name="st")
                        nc.tensor.matmul(
                            st[:, off:],
                            KT[(h, kj // 4)][:, (kj % 4) * 128 : (kj % 4 + 1) * 128],
                            QT[(h, qj)][:, off:],
                            start=True,
                            stop=True,
                        )
                        ad = kj - 4 * qj
                        if mode == "causal" and ad >= 0:
                            nc.vector.tensor_add(
                                st[:, off : off + 128],
                                st[:, off : off + 128],
                                db_sb[:, ad, off : off + 128],
                            )
                        elif mode == "bias":
                            fbt = fb_pool.tile([128, 512], F32, tag="fb", name="fb")
                            nc.sync.dma_start(
                                fbt[:],
                                fbias[
                                    kj * 128 : (kj + 1) * 128,
                                    qj * 512 : (qj + 1) * 512,
                                ],
                            )
                            nc.vector.tensor_add(st[:], st[:], fbt[:])
                        nc.scalar.activation(
                            eq[:, a, off:], st[:, off:],
                            mybir.ActivationFunctionType.Exp,
                        )

                    def _normalize(i):
                        # 1/denom broadcast across partitions on GPSIMD.
                        # gpsimd runs nothing else in attention (memsets moved
                        # to DVE) so the custom-op library stays loaded, and
                        # the PV bank was already freed by the ACT drain, so
                        # this latency only gates the oproj chunks which have
                        # >= 6 steps of slack.
                        rcb = rc_pool.tile([128, 512], F32, tag="rcb", name="rcb")
                        nc.gpsimd.partition_broadcast(rcb[:], RC[i][:], channels=128)
                        ot = ot_pool.tile([128, 512], BF16, tag="ot", name="ot")
                        nc.vector.tensor_mul(ot[:], PVS[i][:], rcb[:])
                        OT[i] = ot

                    def _issue_pvdn(h, kj):
                        off = _off(kj)
                        a = kj % 4
                        eq = EQ[(h, kj // 4)]
                        if kj == 0:
                            PV[h] = psum_pv.tile([128, 512], F32, tag="pv", name="pv")
                            DN[h] = psum_dn.tile([1, 512], F32, tag="dn", name="dn")
                        nc.tensor.matmul(
                            PV[h][:, off:],
                            VT[kj][:, h * 128 : (h + 1) * 128],
                            eq[:, a, off:],
                            start=(kj == 0),
                            stop=(kj == kmax - 1),
                        )
                        if a == 3:
                            # quad denominator: 2 DVE adds + 1 matmul instead
                            # of 4 ones-matmuls
                            quad = kj // 4
                            p2 = ep_pool.tile([128, 2, 512], BF16, tag="p2", name="p2")
                            nc.vector.tensor_add(p2[:], eq[:, 0:2, :], eq[:, 2:4, :])
                            sq = es_pool.tile([128, 512], BF16, tag="sq", name="sq")
                            nc.vector.tensor_add(sq[:], p2[:, 0, :], p2[:, 1, :])
                            nc.tensor.matmul(
                                DN[h][:],
                                ones_sb[:],
                                sq[:],
                                start=(quad == 0),
                                stop=(quad == nquad - 1),
                            )
                        if kj == kmax - 1:
                            # drain PV to SBUF on ACT right at pv-stop: frees
                            # the PV bank early and gives the normalize mul an
                            # SBUF operand (DVE reads at most one PSUM input)
                            pvsb = pvs_pool.tile([128, 512], BF16, tag="pvs", name="pvs")
                            nc.scalar.copy(pvsb[:], PV[h][:])
                            PVS[h] = pvsb
                            rcf = rc_pool.tile([1, 512], F32, tag="rcf", name="rcf")
                            nc.vector.reciprocal_approx_fast(rcf[:], DN[h][:])
                            RC[h] = rcf
                            if h > 0:
                                _normalize(h - 1)
                            if h == HPG - 1:
                                _normalize(h)

                    seq = [(h, kj) for h in range(HPG) for kj in range(kmax)]
                    LOOK = 4
                    chunks = pending_oproj[0] or []
                    pending_oproj[0] = None
                    # previous q-block's o-projection chunks, spread EVENLY
                    # over steps 6..len-1 (start at 6 so the first chunk,
                    # which waits on the previous block's last OT normalize,
                    # never blocks the in-order PE queue ahead of these sts).
                    # Even spacing keeps ACT (exp, ~690ns/tile) from falling
                    # behind in the chunk-free stretch at the end of each
                    # block, where a bare step is only ~550ns of PE work.
                    emit_at = {}
                    if chunks:
                        span = len(seq) - 6
                        for k in range(len(chunks)):
                            emit_at[6 + (k * span) // len(chunks)] = k
                    ci = 0
                    for i, (h, kj) in enumerate(seq):
                        _issue_st(h, kj)
                        if i >= LOOK:
                            _issue_pvdn(*seq[i - LOOK])
                        if i in emit_at and ci < len(chunks):
                            chunks[ci]()
                            ci += 1
                    for i in range(len(seq) - LOOK, len(seq)):
                        _issue_pvdn(*seq[i])
                    while ci < len(chunks):
                        chunks[ci]()
                        ci += 1
                    pending_oproj[0] = make_oproj_chunks(qj, OT, tail=(qj == SJ - 1))
                for ch in pending_oproj[0]:
                    ch()
    nc.compile()
    return nc


_PROGRAM_CACHE = {}


def _get_program(mode):
    if mode not in _PROGRAM_CACHE:
        _PROGRAM_CACHE[mode] = build_program(mode)
    return _PROGRAM_CACHE[mode]


def _detect_mode(attn_mask):
    m = np.asarray(attn_mask).reshape(S, S)
    if (m == np.tril(np.ones((S, S), m.dtype))).all():
        return "causal"
    if (m != 0).all():
        return "full"
    return "bias"


def _diag_bias():
    # [128(p), 4(a), 512(t)]: 0 where 128a+p <= t else -1e30
    a = np.arange(4)[None, :, None]
    p = np.arange(128)[:, None, None]
    t = np.arange(512)[None, None, :]
    return np.where(128 * a + p <= t, 0.0, NEG).astype(np.float32)


def _bf16(a):
    return np.ascontiguousarray(a).astype(NPBF16)


def _prep_w(wT):
    # [H, DLOC] -> [128, FT, DLOC] with [p, ft, d] = wT[ft*128+p, d]
    return np.ascontiguousarray(wT.reshape(FT, 128, DLOC).transpose(1, 0, 2))


def kernel(**inputs) -> np.ndarray:
    from concourse.bass_utils import run_bass_kernel_spmd

    x = np.asarray(inputs["x"], np.float32)
    fcos = np.asarray(inputs["fcos"], np.float32)
    fsin = np.asarray(inputs["fsin"], np.float32)
    Wq, bq = np.asarray(inputs["Wq"], np.float32), np.asarray(inputs["bq"], np.float32)
    Wk, bk = np.asarray(inputs["Wk"], np.float32), np.asarray(inputs["bk"], np.float32)
    Wv, bv = np.asarray(inputs["Wv"], np.float32), np.asarray(inputs["bv"], np.float32)
    Wo, bo = np.asarray(inputs["Wo"], np.float32), np.asarray(inputs["bo"], np.float32)
    attn_mask = inputs["attn_mask"]

    mode = _detect_mode(attn_mask)
    nc = _get_program(mode)

    sc = 1.0 / math.sqrt(HD)
    sinF = fsin.T.copy()  # [HD, S]
    sinF[0::2, :] *= -1.0  # rope[2i] = t[2i]*cos - t[2i+1]*sin
    shared = {
        "cosT": _bf16(fcos.T),
        "sinT": _bf16(sinF),
    }
    if mode == "causal":
        shared["dbp"] = _diag_bias()
    elif mode == "bias":
        m = np.asarray(attn_mask).reshape(S, S)
        shared["fbias"] = np.ascontiguousarray(
            np.where(m.T == 0, NEG, 0.0).astype(np.float32)
        )

    in_maps = []
    for c in range(NCORES):
        b, hg = divmod(c, HG)
        rows = slice(DLOC * hg, DLOC * (hg + 1))
        xT = x[b].T  # [H, S]
        # [128, SJ, FT, 512]: [p, sj, ft, s] = xT[ft*128+p, sj*512+s]
        xprep = xT.reshape(FT, 128, SJ, 512).transpose(1, 2, 0, 3)
        woT = Wo[:, rows].T  # [DLOC, H]
        wo_prep = woT.reshape(HPG, 128, H).transpose(1, 0, 2)
        in_maps.append(
            {
                "xp": _bf16(xprep),
                "wqp": _bf16(_prep_w((Wq[rows] * sc).T)),
                "wkp": _bf16(_prep_w(Wk[rows].T)),
                "wvp": _bf16(_prep_w(Wv[rows].T)),
                "wop": _bf16(wo_prep),
                "bqT": np.ascontiguousarray((bq[rows] * sc).reshape(HPG, 128).T),
                "bkT": np.ascontiguousarray(bk[rows].reshape(HPG, 128).T),
                **shared,
            }
        )

    trace = bool(int(os.environ.get("KERNEL_TRACE", "0")))
    res = run_bass_kernel_spmd(nc, in_maps, list(range(NCORES)), trace=trace)
    if trace and res.exec_time_ns is not None:
        print(f"HW exec time: {res.exec_time_ns} ns")
        globals()["LAST_EXEC_NS"] = res.exec_time_ns
        globals()["LAST_RESULTS"] = res

    out = np.zeros((B, S, H), np.float32)
    for c in range(NCORES):
        yt = np.asarray(res.results[c]["y2"]).astype(np.float32)  # [SJ,4,128,H]
        out[c // HG] += yt.reshape(S, H)
    # bv is folded here: attn weights sum to 1, so attn(v + bv) = attn(v) + bv
    # and the output projection adds the constant row bv @ Wo.T exactly.
    out += bo + bv @ Wo.T
    return out
